# revision 7
# baseline (speedup 1.0000x reference)
"""AdaptiveRankSemiseparableLayer on 8 trn2 NeuronCores.

Reference semantics (B=4, L=4096, D=1024, R=32, GH=256):
    h     = relu(x @ gate_w1 + gate_b1)            # (B,L,GH)
    gate  = sigmoid(h @ gate_w2 + gate_b2)         # (B,L,R)
    U     = x @ U_w ;  V = x @ V_w                 # (B,L,R)
    S     = cumsum(V, axis=1)                      # causal scan
    y_g   = (gate*U*S) @ out_w + out_b             # (B,L,D)
    t_out = depthwise_conv1d(x, conv_w, k=3, pad 1)
    out   = t_out + y_g

Sharding: 8 shards of 2048 contiguous tokens (2 per batch).  The cumsum
carry of the first half of each batch reaches the second half via a tiny
AllGather of per-core V-totals + a per-core selection-mask matmul.

Per-core device layout: everything transposed (d / gh / r on SBUF
partitions, tokens on the free dim).  x^T arrives via DMA-transpose of a
host-precast bf16 shard with one halo token on each side.  The 3-tap
depthwise conv: tap(x[t-1]) on DVE tensor_scalar, taps(x[t], x[t+1]) as
diagonal-matrix matmuls accumulated into the same PSUM tile as the R->D
output projection.  The final op per chunk fuses out_b add + PSUM add on
DVE and emits bf16; the host unshards/transposes back to f32 (B,L,D).
"""

import os

import numpy as np
import ml_dtypes

from concourse import bacc, mybir, tile
from concourse.bass_utils import run_bass_kernel_spmd

F32 = mybir.dt.float32
BF16 = mybir.dt.bfloat16
AX = mybir.AluOpType
AF = mybir.ActivationFunctionType
BF16NP = ml_dtypes.bfloat16

B, L, D, R, GH = 4, 4096, 1024, 32, 256
NCORES = 8
TOK = 2048          # tokens per core
G = 512             # token group (matmul rhs free size)
NG = TOK // G       # 4 groups
NCH = D // 128      # 8 d-chunks
XROWS = 2064        # 1 halo + 2048 + 1 halo + pad to %16

# tap placement knobs (tuned against the profile):
#   tap x[t-1] (w0): DVE tensor_scalar (aligned, 4x mode)
#   tap x[t]   (w1): PE diag matmul
#   tap x[t+1] (w2): PE diag for chunks < TAP3_PE_CHUNKS else DVE STT
TAP3_PE_CHUNKS = int(os.environ.get("TAP3_PE_CHUNKS", "8"))


def _build(weights_np):
    nc = bacc.Bacc(None, target_bir_lowering=False, debug=False)

    x_ext = nc.declare_dram_parameter("x", [XROWS, D], BF16, isOutput=False)
    cmask_ext = nc.declare_dram_parameter("cmask", [NCORES, 1], F32, isOutput=False)
    y_ext = nc.declare_dram_parameter("y", [NG, 128, NCH, G], BF16, isOutput=True)

    cw = {k: nc.inline_tensor(v, name=k) for k, v in weights_np.items()}

    with tile.TileContext(nc) as tc:
        with (
            tc.tile_pool(name="wsb", bufs=1) as wsb,
            tc.tile_pool(name="xsb", bufs=1) as xsb,
            tc.tile_pool(name="hsb", bufs=3) as hsb,
            tc.tile_pool(name="ssb", bufs=2) as ssb,
            tc.tile_pool(name="ysb", bufs=2) as ysb,
            tc.tile_pool(name="hps", bufs=2, space="PSUM") as hps,
            tc.tile_pool(name="uvps", bufs=2, space="PSUM") as uvps,
            tc.tile_pool(name="gps", bufs=1, space="PSUM") as gps,
            tc.tile_pool(name="yps", bufs=2, space="PSUM") as yps,
            tc.tile_pool(name="tinyps", bufs=1, space="PSUM") as tinyps,
            tc.tile_pool(name="dram", bufs=1, space="DRAM") as dram,
        ):
            # ---- weights -> SBUF ----
            w1sb = wsb.tile([128, NCH * GH], BF16, name="w1sb")
            nc.sync.dma_start(out=w1sb[:, :], in_=cw["w1"][:, :])
            uvwsb = wsb.tile([128, NCH * 2 * R], BF16, name="uvwsb")
            nc.sync.dma_start(out=uvwsb[:, :], in_=cw["uvw"][:, :])
            w2sb = wsb.tile([128, 2 * R], BF16, name="w2sb")
            nc.sync.dma_start(out=w2sb[:, :], in_=cw["w2"][:, :])
            outwsb = wsb.tile([R, D], BF16, name="outwsb")
            nc.sync.dma_start(out=outwsb[:, :], in_=cw["outw"][:, :])
            diag1sb = wsb.tile([128, NCH * 128], BF16, name="diag1sb")
            nc.sync.dma_start(out=diag1sb[:, :], in_=cw["diag1"][:, :])
            diag2sb = wsb.tile([128, NCH * 128], BF16, name="diag2sb")
            nc.sync.dma_start(out=diag2sb[:, :], in_=cw["diag2"][:, :])
            smallsb = wsb.tile([128, 28], F32, name="smallsb")
            nc.sync.dma_start(out=smallsb[:, :], in_=cw["small"][:, :])
            # small cols: 0:8 w0col, 8:16 out_b chunks, 16:24 w2col, 24:26 b1, 26 b2
            w0col = smallsb[:, 0:8]
            outb = smallsb[:, 8:16]
            w2col = smallsb[:, 16:24]
            b1 = smallsb[:, 24:26]
            b2 = smallsb[0:R, 26:27]
            cmsb = wsb.tile([NCORES, 1], F32, name="cmsb")
            nc.sync.dma_start(out=cmsb[:, :], in_=cmask_ext[:, :])
            cmbf = wsb.tile([NCORES, 1], BF16, name="cmbf")
            nc.vector.tensor_copy(cmbf[:, :], cmsb[:, :])

            # ---- x^T via DMA transpose (per d-chunk, two row-blocks) ----
            xT = []
            for c in range(NCH):
                t = xsb.tile([128, XROWS], BF16, name=f"xT{c}")
                nc.sync.dma_start(
                    out=t[:, 0:1040], in_=x_ext[0:1040, c * 128:(c + 1) * 128],
                    transpose=True,
                )
                nc.sync.dma_start(
                    out=t[:, 1040:XROWS], in_=x_ext[1040:XROWS, c * 128:(c + 1) * 128],
                    transpose=True,
                )
                xT.append(t)

            # ---- conv tap x[t-1]: t_sb[c] = w0[d] * x^T[:, 0:2048] ----
            t_sb = []
            for c in range(NCH):
                t = xsb.tile([128, TOK], BF16, name=f"tsb{c}")
                nc.vector.tensor_scalar_mul(t[:, :], xT[c][:, 0:TOK], w0col[:, c:c + 1])
                t_sb.append(t)

            # ---- carry-independent per-group work ----
            S_sb = ssb.tile([R, TOK], F32, name="S_sb", bufs=1)
            junk = ssb.tile([R, 1], F32, name="junk", bufs=1)
            u_tiles, h_tiles, gate_tiles = [], [], []
            for g in range(NG):
                lo = 1 + g * G
                # U,V projections (fused lhsT [128, 64] per chunk)
                uv = uvps.tile([2 * R, G], F32, name="uv")
                for c in range(NCH):
                    nc.tensor.matmul(
                        uv[:, :], uvwsb[:, c * 64:(c + 1) * 64],
                        xT[c][:, lo:lo + G], start=(c == 0), stop=(c == NCH - 1),
                    )
                # causal scan of V; the scan inst has no sync-wait slots, so a
                # same-engine touch op absorbs the PE dependency first.
                nc.vector.tensor_copy(junk[:, :], uv[R:2 * R, 0:1])
                init = 0.0 if g == 0 else S_sb[:, g * G - 1:g * G]
                nc.vector.tensor_tensor_scan(
                    S_sb[:, g * G:(g + 1) * G], uv[R:2 * R, :],
                    junk[:, 0:1].broadcast_to((R, G)), init, AX.add, AX.bypass,
                )
                # U^T out of PSUM so the uv slot frees before the carry gate
                u_sb = ssb.tile([R, G], BF16, name="u_sb", bufs=4)
                nc.vector.tensor_copy(u_sb[:, :], uv[0:R, :])
                u_tiles.append(u_sb)
                # gate MLP
                htg = []
                for j in range(2):
                    hp = hps.tile([128, G], F32, name="hp")
                    for c in range(NCH):
                        nc.tensor.matmul(
                            hp[:, :], w1sb[:, (c * 2 + j) * 128:(c * 2 + j + 1) * 128],
                            xT[c][:, lo:lo + G], start=(c == 0), stop=(c == NCH - 1),
                        )
                    hs = hsb.tile([128, G], BF16, name="hs")
                    nc.vector.tensor_scalar(
                        hs[:, :], hp[:, :], b1[:, j:j + 1], 0.0, AX.add, AX.max
                    )
                    htg.append(hs)
                h_tiles.append(htg)
                gp = gps.tile([R, G], F32, name="gp")
                for j in range(2):
                    nc.tensor.matmul(
                        gp[:, :], w2sb[:, j * R:(j + 1) * R], htg[j][:, :],
                        start=(j == 0), stop=(j == 1),
                    )
                gate = ssb.tile([R, G], BF16, name="gate", bufs=4)
                nc.scalar.activation(gate[:, :], gp[:, :], AF.Sigmoid, bias=b2, scale=1.0)
                gate_tiles.append(gate)

            # ---- cross-core carry: AllGather per-core V totals ----
            cc_in = dram.tile([R, 1], F32, name="cc_in")
            cc_out = dram.tile([NCORES, R], F32, name="cc_out", addr_space="Shared")
            nc.scalar.dma_start(out=cc_in[:, :], in_=S_sb[:, TOK - 1:TOK])
            nc.gpsimd.collective_compute(
                "AllGather", AX.bypass, ins=[cc_in.opt()], outs=[cc_out.opt()],
                replica_groups=[list(range(NCORES))],
            )
            gath = wsb.tile([NCORES, R], F32, name="gath")
            nc.scalar.dma_start(out=gath[:, :], in_=cc_out[:, :])
            gathbf = wsb.tile([NCORES, R], BF16, name="gathbf")
            nc.vector.tensor_copy(gathbf[:, :], gath[:, :])
            carry_ps = tinyps.tile([R, 1], F32, name="carry_ps")
            nc.tensor.matmul(carry_ps[:, :], gathbf[:, :], cmbf[:, :], start=True, stop=True)
            carry = wsb.tile([R, 1], F32, name="carry")
            nc.vector.tensor_copy(carry[:, :], carry_ps[:, :])

            # ---- carry-dependent tail ----
            for g in range(NG):
                lo = 1 + g * G
                u_sb, gate = u_tiles[g], gate_tiles[g]
                t1 = hsb.tile([R, G], F32, name="t1")
                nc.vector.tensor_mul(t1[:, :], gate[:, :], u_sb[:, :])
                glob = hsb.tile([R, G], BF16, name="glob")
                nc.vector.scalar_tensor_tensor(
                    glob[:, :], S_sb[:, g * G:(g + 1) * G], carry[:, 0:1], t1[:, :],
                    AX.add, AX.mult,
                )
                y_sb = ysb.tile([128, NCH * G], BF16, name="y_sb")
                for c in range(NCH):
                    yp = yps.tile([128, G], F32, name="yp")
                    nc.tensor.matmul(
                        yp[:, :], diag1sb[:, c * 128:(c + 1) * 128],
                        xT[c][:, lo:lo + G], start=True, stop=False,
                    )
                    if c < TAP3_PE_CHUNKS:
                        nc.tensor.matmul(
                            yp[:, :], diag2sb[:, c * 128:(c + 1) * 128],
                            xT[c][:, lo + 1:lo + G + 1], start=False, stop=False,
                        )
                    nc.tensor.matmul(
                        yp[:, :], outwsb[:, c * 128:(c + 1) * 128], glob[:, :],
                        start=False, stop=True,
                    )
                    if c < TAP3_PE_CHUNKS:
                        nc.vector.scalar_tensor_tensor(
                            y_sb[:, c * G:(c + 1) * G], t_sb[c][:, g * G:(g + 1) * G],
                            outb[:, c:c + 1], yp[:, :], AX.add, AX.add,
                        )
                    else:
                        tmp = hsb.tile([128, G], BF16, name="tmp")
                        nc.vector.scalar_tensor_tensor(
                            tmp[:, :], xT[c][:, lo + 1:lo + G + 1], w2col[:, c:c + 1],
                            t_sb[c][:, g * G:(g + 1) * G], AX.mult, AX.add,
                        )
                        nc.vector.scalar_tensor_tensor(
                            y_sb[:, c * G:(c + 1) * G], tmp[:, :], outb[:, c:c + 1],
                            yp[:, :], AX.add, AX.add,
                        )
                nc.scalar.dma_start(out=y_ext[g, :, :, :], in_=y_sb[:, :])

    nc.finalize()
    return nc


def _prep_weights(gate_w1, gate_b1, gate_w2, gate_b2, U_w, V_w, conv_w, out_w, out_b):
    bf = lambda a: np.ascontiguousarray(a).astype(BF16NP)
    f32 = lambda a: np.ascontiguousarray(a).astype(np.float32)
    w1 = np.concatenate([gate_w1[c * 128:(c + 1) * 128, :] for c in range(NCH)], axis=1)
    uvw = np.concatenate(
        [np.concatenate([U_w[c * 128:(c + 1) * 128, :], V_w[c * 128:(c + 1) * 128, :]], axis=1)
         for c in range(NCH)], axis=1)
    w2 = np.concatenate([gate_w2[j * 128:(j + 1) * 128, :] for j in range(2)], axis=1)
    diag1 = np.concatenate(
        [np.diag(conv_w[c * 128:(c + 1) * 128, 1]) for c in range(NCH)], axis=1)
    diag2 = np.concatenate(
        [np.diag(conv_w[c * 128:(c + 1) * 128, 2]) for c in range(NCH)], axis=1)
    small = np.zeros((128, 28), np.float32)
    small[:, 0:8] = conv_w[:, 0].reshape(NCH, 128).T
    small[:, 8:16] = out_b.reshape(NCH, 128).T
    small[:, 16:24] = conv_w[:, 2].reshape(NCH, 128).T
    small[:, 24:26] = gate_b1.reshape(2, 128).T
    small[0:R, 26] = gate_b2
    return {
        "w1": bf(w1), "uvw": bf(uvw), "w2": bf(w2), "outw": bf(out_w),
        "diag1": bf(diag1), "diag2": bf(diag2), "small": f32(small),
    }


def _shard_x(x):
    """Per-core bf16 shard [XROWS, D]: 1 halo row, 2048 tokens, 1 halo, pad."""
    shards = []
    for c in range(NCORES):
        b, h = c // 2, c % 2
        t0 = h * TOK
        s = np.zeros((XROWS, D), np.float32)
        lo, hi = t0 - 1, t0 + TOK + 1
        src_lo, src_hi = max(lo, 0), min(hi, L)
        s[src_lo - lo:src_lo - lo + (src_hi - src_lo), :] = x[b, src_lo:src_hi, :]
        shards.append(s.astype(BF16NP))
    return shards


def _run(inputs, trace=False, tmpdir=None):
    x = np.asarray(inputs["x"], np.float32)
    weights = _prep_weights(
        *[np.asarray(inputs[k], np.float32) for k in
          ("gate_w1", "gate_b1", "gate_w2", "gate_b2", "U_w", "V_w",
           "conv_w", "out_w", "out_b")])
    nc = _build(weights)
    shards = _shard_x(x)
    in_maps = []
    for c in range(NCORES):
        cm = np.zeros((NCORES, 1), np.float32)
        if c % 2 == 1:
            cm[c - 1, 0] = 1.0
        in_maps.append({"x": shards[c], "cmask": cm})
    res = run_bass_kernel_spmd(
        nc, in_maps, core_ids=list(range(NCORES)), trace=trace, tmpdir=tmpdir
    )
    out = np.empty((B, L, D), np.float32)
    for c in range(NCORES):
        b, h = c // 2, c % 2
        yc = np.asarray(res.results[c]["y"]).astype(np.float32)
        # [g, p, ch, t] -> [(g t), (ch p)]
        yc = yc.transpose(0, 3, 2, 1).reshape(TOK, D)
        out[b, h * TOK:(h + 1) * TOK, :] = yc
    return out, res


def kernel(**inputs) -> np.ndarray:
    out, _ = _run(inputs)
    return out
